# revision 19
# baseline (speedup 1.0000x reference)
"""Multi-head attention forward on 8 Trainium2 NeuronCores (Bass/Tile).

Problem: B=2, N=2048, D=1024, H=16 heads of dh=64, fp32.

Sharding: tensor-parallel over heads — core c owns heads {2c, 2c+1} and both
batches for projections + attention; an on-device AllToAll then re-shards by
token so each core computes the output projection (full Wo) for its 512-token
slice with no reduction.

Layouts: all activations travel as [feature, token] ("transposed"), so every
matmul contraction lands on the partition axis:
  qT/kT/vT [128, 4096]  (rows 0-63 head A dims, 64-127 head B dims)
  scoresT[m, n] = kT.T @ qT   (softmax axis m = partitions)
  exp via ScalarE (no max subtraction: scores ~ N(0,1), exp is safe in fp32)
  attn@v: lhsT = v_aug [m, 65] (v transposed back per 128-chunk via PE
  transpose, with a ones column appended) so PSUM row 64 accumulates the
  softmax denominators for free.
  normalization: reciprocal of denom row, broadcast across partitions with a
  one-hot selector matmul, applied on VectorE.

Matmuls run in float32r (TF32-like, ~1.5e-4 rel err, full PE rate at free
dim >= 256). fp32 inputs are DMA'd directly into float32r tiles (legal when
the DRAM tensor is declared float32r).
"""
from contextlib import ExitStack

import numpy as np

import concourse.bass as bass
import concourse.tile as tile
from concourse import bacc, mybir
from concourse.bass_utils import run_bass_kernel_spmd
from concourse.masks import make_identity

F32 = mybir.dt.float32
F32R = mybir.dt.float32r

B, N, D, H, DH = 2, 2048, 1024, 16, 64
W = 8                    # cores
TOK = B * N              # 4096 flattened tokens
TPC = TOK // W           # 512 tokens per core after re-shard
HPC = H // W             # 2 heads per core

_CACHE = {}


def build_bass():
    nc = bacc.Bacc("TRN2", target_bir_lowering=False)

    xT_d = nc.declare_dram_parameter("xT", [D, TOK], F32R, isOutput=False)
    wq_d = nc.declare_dram_parameter("wq", [D, 128], F32R, isOutput=False)
    wk_d = nc.declare_dram_parameter("wk", [D, 128], F32R, isOutput=False)
    wv_d = nc.declare_dram_parameter("wv", [D, 128], F32R, isOutput=False)
    wo_d = nc.declare_dram_parameter("wo", [D, D], F32R, isOutput=False)
    bqkv_d = nc.declare_dram_parameter("bqkv", [128, 3], F32, isOutput=False)
    out_d = nc.declare_dram_parameter("out", [TPC, D], F32, isOutput=True)

    a2a_in1 = nc.dram_tensor("a2a_in1", [W, 128, 256], F32R)
    a2a_out1 = nc.dram_tensor("a2a_out1", [W, 128, 256], F32R)
    a2a_in2a = nc.dram_tensor("a2a_in2a", [W, 128, 128], F32R)
    a2a_out2a = nc.dram_tensor("a2a_out2a", [W, 128, 128], F32R)
    a2a_in2b = nc.dram_tensor("a2a_in2b", [W, 128, 128], F32R)
    a2a_out2b = nc.dram_tensor("a2a_out2b", [W, 128, 128], F32R)

    KC = D // 128        # contraction chunks for projections
    TC = TOK // 512      # 512-token chunks (8)
    MCB = N // 128       # m-chunks per batch (16)

    with tile.TileContext(nc) as tc, ExitStack() as ctx:
        sb1 = ctx.enter_context(tc.tile_pool(name="sb1", bufs=1))
        sbe = ctx.enter_context(tc.tile_pool(name="sbe", bufs=2))
        stage1 = ExitStack()
        sbw = stage1.enter_context(tc.tile_pool(name="sbw", bufs=1))
        sbx = stage1.enter_context(tc.tile_pool(name="sbx", bufs=2))
        ps_pj = stage1.enter_context(tc.tile_pool(name="ps_pj", bufs=2, space="PSUM"))

        # ---------- constants ----------
        ident_f = sb1.tile([128, 128], F32, tag="ident_f")
        make_identity(nc, ident_f[:])
        ident = sb1.tile([128, 128], F32R, tag="ident")
        nc.vector.tensor_copy(ident[:], ident_f[:])

        ones_f = sb1.tile([128, 1], F32, tag="ones_f")
        nc.vector.memset(ones_f[:], 1.0)
        ones_r = sb1.tile([128, 1], F32R, tag="ones_r")
        nc.vector.tensor_copy(ones_r[:], ones_f[:])

        sel = sb1.tile([128, 128], F32, tag="sel")
        nc.vector.memset(sel[:], 0.0)
        nc.vector.memset(sel[0:1, 0:64], 1.0)
        nc.vector.memset(sel[64:65, 64:128], 1.0)

        bias = sb1.tile([128, 3], F32, tag="bias")
        nc.sync.dma_start(bias[:], bqkv_d[:])

        zeros_f = sb1.tile([128, 512], F32, tag="zeros_f")
        nc.vector.memset(zeros_f[:], 0.0)
        zeros_r = sb1.tile([128, 512], F32R, tag="zeros_r")
        nc.vector.tensor_copy(zeros_r[:], zeros_f[:])

        # ---------- weights ----------
        wq = sbw.tile([128, KC, 128], F32R, tag="wq")
        wk = sbw.tile([128, KC, 128], F32R, tag="wk")
        wv = sbw.tile([128, KC, 128], F32R, tag="wv")
        for k in range(KC):
            nc.sync.dma_start(wq[:, k, :], wq_d[bass.ts(k, 128), :])

        # ---------- stage 1: projections (qT, kT resident; v -> v_aug) ----------
        qT = sb1.tile([128, TOK], F32R, tag="qT")
        # per-head kT, zero-padded to K=128 so score matmuls use the full
        # PE array (half-array shapes leave the clock gate cold)
        kT0p = sb1.tile([128, TOK], F32R, tag="kT0p")
        kT1p = sb1.tile([128, TOK], F32R, tag="kT1p")
        v_aug = sb1.tile([128, 2 * MCB, 130], F32R, tag="v_aug")

        for tp2 in range(TC // 2):
            ta, tb = 2 * tp2, 2 * tp2 + 1
            xta = sbx.tile([128, KC, 512], F32R, tag="xta")
            xtb = sbx.tile([128, KC, 512], F32R, tag="xtb")
            for k in range(KC):
                nc.sync.dma_start(xta[:, k, :], xT_d[bass.ts(k, 128), bass.ts(ta, 512)])
            for k in range(KC):
                nc.sync.dma_start(xtb[:, k, :], xT_d[bass.ts(k, 128), bass.ts(tb, 512)])
            if tp2 == 0:
                for k in range(KC):
                    nc.sync.dma_start(wk[:, k, :], wk_d[bass.ts(k, 128), :])
                    nc.sync.dma_start(wv[:, k, :], wv_d[bass.ts(k, 128), :])

            tsla, tslb = bass.ts(ta, 512), bass.ts(tb, 512)
            pja = ps_pj.tile([128, 512], F32, tag="pj0")
            pjb = ps_pj.tile([128, 512], F32, tag="pj1")
            for k in range(KC):
                nc.tensor.matmul(pja[:], wq[:, k, :], xta[:, k, :],
                                 start=(k == 0), stop=(k == KC - 1))
                nc.tensor.matmul(pjb[:], wq[:, k, :], xtb[:, k, :],
                                 start=(k == 0), stop=(k == KC - 1))
            nc.vector.tensor_scalar_add(qT[:, tsla], pja[:], bias[:, 0:1])
            nc.vector.tensor_scalar_add(qT[:, tslb], pjb[:], bias[:, 0:1])

            pja = ps_pj.tile([128, 512], F32, tag="pj0")
            pjb = ps_pj.tile([128, 512], F32, tag="pj1")
            for k in range(KC):
                nc.tensor.matmul(pja[:], wk[:, k, :], xta[:, k, :],
                                 start=(k == 0), stop=(k == KC - 1))
                nc.tensor.matmul(pjb[:], wk[:, k, :], xtb[:, k, :],
                                 start=(k == 0), stop=(k == KC - 1))
            for tsl, pj in ((tsla, pja), (tslb, pjb)):
                nc.vector.tensor_scalar_add(kT0p[0:64, tsl], pj[0:64, :], bias[0:64, 1:2])
                nc.vector.tensor_scalar_add(kT1p[64:128, tsl], pj[64:128, :], bias[64:128, 1:2])
                nc.vector.tensor_copy(kT0p[64:128, tsl], zeros_r[64:128, :])
                nc.vector.tensor_copy(kT1p[0:64, tsl], zeros_r[0:64, :])

            pja = ps_pj.tile([128, 512], F32, tag="pj0")
            pjb = ps_pj.tile([128, 512], F32, tag="pj1")
            for k in range(KC):
                nc.tensor.matmul(pja[:], wv[:, k, :], xta[:, k, :],
                                 start=(k == 0), stop=(k == KC - 1))
                nc.tensor.matmul(pjb[:], wv[:, k, :], xtb[:, k, :],
                                 start=(k == 0), stop=(k == KC - 1))
            vts = []
            for t, pj in ((ta, pja), (tb, pjb)):
                vt = sbx.tile([128, 512], F32R, tag=f"vt{t % 2}")
                nc.vector.tensor_scalar_add(vt[:], pj[:], bias[:, 2:3])
                vts.append((t, vt))
            # transpose v into v_aug rows (4 m-chunks per 512-token group)
            for t, vt in vts:
                for i in range(4):
                    gm = 4 * t + i
                    tp = ps_pj.tile([128, 128], F32R, tag="tp")
                    nc.tensor.transpose(tp[:], vt[:, bass.ts(i, 128)], ident[:])
                    nc.vector.tensor_copy(v_aug[:, gm, 0:64], tp[:, 0:64])
                    nc.vector.tensor_copy(v_aug[:, gm, 65:129], tp[:, 64:128])
                    nc.vector.tensor_copy(v_aug[:, gm, 64:65], ones_r[:])
                    nc.vector.tensor_copy(v_aug[:, gm, 129:130], ones_r[:])

        stage1.close()
        sb3 = ctx.enter_context(tc.tile_pool(name="sb3", bufs=1))
        wo = sb3.tile([128, KC, D], F32R, tag="wo")
        for k in range(KC):
            nc.sync.dma_start(wo[:, k, :], wo_d[bass.ts(k, 128), :])
        # ---------- stage 2: attention ----------
        stage2 = ExitStack()
        ps_sc = stage2.enter_context(tc.tile_pool(name="ps_sc", bufs=1, space="PSUM"))
        ps_ha = stage2.enter_context(tc.tile_pool(name="ps_ha", bufs=1, space="PSUM"))
        heads = sb1.tile([128, TOK], F32R, tag="heads")
        rcp = sb1.tile([128, 1024], F32, tag="rcp")
        nc.vector.memset(rcp[:], 0.0)
        rsm = sb1.tile([128, 16], F32, tag="rsm")

        def emit_a2a(a_in, a_out, col0, width):
            for j in range(W):
                nc.sync.dma_start(a_in[j], heads[:, bass.ds(col0 + width * j, width)])
            nc.gpsimd.collective_compute(
                "AllToAll",
                mybir.AluOpType.bypass,
                ins=[a_in[:]],
                outs=[a_out[:]],
                replica_groups=[list(range(W))],
            )

        def emit_normalize(pend, bc_pool=None, bc_tag="sc0"):
            # selector matmul broadcasts 1/denom across partitions, then
            # VectorE applies it; emitted one window late so the PE-queue
            # stall on the reciprocal DMA round-trip hides inside the next
            # window's matmul stream.
            hs0, hs1, ptok0, pb, pnh = pend
            pool = bc_pool if bc_pool is not None else ps_sc
            for q4 in range(2):
                psl = bass.ts(q4, 512)
                bc = pool.tile([128, 512], F32, tag=bc_tag)
                nc.tensor.matmul(bc[:], sel[:], rcp[:, psl], start=True, stop=True)
                bc_s = sbe.tile([128, 512], F32, tag="bc_s", bufs=1)
                nc.vector.tensor_copy(bc_s[:], bc[:])
                hsl = bass.ds(ptok0 + 512 * q4, 512)
                nc.vector.tensor_mul(heads[0:64, hsl], hs0[0:64, psl], bc_s[0:64, :])
                nc.vector.tensor_mul(heads[64:128, hsl], hs1[64:128, psl], bc_s[64:128, :])
            if (pb, pnh) == (0, 1):
                emit_a2a(a2a_in1, a2a_out1, 0, 256)       # batch-0 heads
            elif (pb, pnh) == (1, 0):
                emit_a2a(a2a_in2a, a2a_out2a, 2048, 128)  # batch-1 first half

        pending = None
        for b in range(B):
            for nh in range(2):                   # 1024-token n-window
                tok0 = 2048 * b + 1024 * nh
                ha0 = ps_ha.tile([65, 1024], F32, tag="ha0")
                ha1 = ps_ha.tile([65, 1024], F32, tag="ha1")
                # software pipeline: attn@v for m-chunk mc-1 runs alongside
                # scores/exp for mc, so PE never waits on a fresh exp.
                prev = None
                for mc in range(MCB):
                    gm = MCB * b + mc
                    msl = bass.ts(gm, 128)
                    sc0 = ps_sc.tile([128, 1024], F32, tag="sc0")
                    sc1 = ps_sc.tile([128, 1024], F32, tag="sc1")
                    for q4 in range(2):
                        nsl = bass.ds(tok0 + 512 * q4, 512)
                        psl = bass.ts(q4, 512)
                        nc.tensor.matmul(sc0[:, psl], kT0p[:, msl], qT[:, nsl],
                                         start=True, stop=True)
                        nc.tensor.matmul(sc1[:, psl], kT1p[:, msl], qT[:, nsl],
                                         start=True, stop=True)
                    if prev is not None:
                        pe0, pe1, pgm = prev
                        for q4 in range(2):
                            psl = bass.ts(q4, 512)
                            nc.tensor.matmul(ha0[:, psl], v_aug[:, pgm, 0:65],
                                             pe0[:, psl], start=(pgm % MCB == 0), stop=False)
                            nc.tensor.matmul(ha1[:, psl], v_aug[:, pgm, 65:130],
                                             pe1[:, psl], start=(pgm % MCB == 0), stop=False)
                    e0 = sbe.tile([128, 1024], F32R, tag="e0")
                    e1 = sbe.tile([128, 1024], F32R, tag="e1")
                    nc.scalar.activation(e0[:], sc0[:], mybir.ActivationFunctionType.Exp)
                    nc.scalar.activation(e1[:], sc1[:], mybir.ActivationFunctionType.Exp)
                    prev = (e0, e1, gm)
                    if mc == 3 and pending is not None:
                        emit_normalize(pending)
                        pending = None
                # epilogue: last m-chunk's attn@v
                pe0, pe1, pgm = prev
                for q4 in range(2):
                    psl = bass.ts(q4, 512)
                    nc.tensor.matmul(ha0[:, psl], v_aug[:, pgm, 0:65], pe0[:, psl],
                                     start=False, stop=True)
                    nc.tensor.matmul(ha1[:, psl], v_aug[:, pgm, 65:130], pe1[:, psl],
                                     start=False, stop=True)

                # free the ha PSUM banks quickly: copy to SBUF, then compute
                # reciprocals off the PE queue (VectorE + DMA reshape).
                hs0 = sbe.tile([65, 1024], F32, tag="hs0", bufs=1)
                hs1 = sbe.tile([128, 1024], F32, tag="hs1", bufs=1)
                nc.vector.tensor_copy(hs0[:], ha0[:])
                nc.vector.tensor_copy(hs1[64:128, :], ha1[0:64, :])
                nc.vector.tensor_copy(rcp[32:33, :], hs0[64:65, :])
                nc.vector.tensor_copy(rcp[96:97, :], ha1[64:65, :])
                # issue the reshape DMAs from ScalarE's queue so the
                # scheduler can't order them behind collective-gated DMAs
                # on the Sync queue (rows {32,96} -> [128,16] in one shot)
                nc.scalar.dma_start(rsm[:], rcp[32:97:64, :])
                rsr = sbe.tile([128, 16], F32, tag="rsr")
                nc.vector.reciprocal(rsr[:], rsm[:])
                nc.scalar.dma_start(rcp[0:65:64, :], rsr[:])
                pending = (hs0, hs1, tok0, b, nh)

        stage2.close()
        # ---------- stage 3: output projection on re-sharded tokens ----------
        # hT is split into one tile per collective so the out-proj matmuls for
        # already-received tokens don't falsely wait on later collectives.
        ps_op = ctx.enter_context(tc.tile_pool(name="ps_op", bufs=2, space="PSUM"))
        hT_a = sb3.tile([128, W, 256], F32R, tag="hT_a")
        hT_b = sb3.tile([128, W, 128], F32R, tag="hT_b")
        hT_c = sb3.tile([128, W, 128], F32R, tag="hT_c")

        def emit_outproj(tq, src_t, col0):
            for dc in range(2):
                op = ps_op.tile([128, 512], F32, tag="op")
                for k in range(KC):
                    nc.tensor.matmul(op[:], src_t[:, k, bass.ds(col0, 128)],
                                     wo[:, k, bass.ts(dc, 512)],
                                     start=(k == 0), stop=(k == KC - 1))
                ot = sb3.tile([128, 512], F32, tag="ot", bufs=2)
                # ScalarE copy: VectorE's queue is busy with the normalize
                # chain at the tail and would head-of-line block these
                nc.scalar.activation(ot[:], op[:], mybir.ActivationFunctionType.Copy)
                nc.sync.dma_start(out_d[bass.ts(tq, 128), bass.ts(dc, 512)], ot[:])

        # tokens already received (batch 0 + batch-1 first half) project while
        # the final window's normalize chain and last collective run
        for j in range(W):
            nc.sync.dma_start(hT_a[:, j, :], a2a_out1[j])
        emit_outproj(0, hT_a, 0)
        emit_outproj(1, hT_a, 128)
        for j in range(W):
            nc.gpsimd.dma_start(hT_b[:, j, :], a2a_out2a[j])
        emit_outproj(2, hT_b, 0)

        emit_normalize(pending, bc_pool=ps_op, bc_tag="op")
        pending = None
        emit_a2a(a2a_in2b, a2a_out2b, 3072, 128)          # batch-1 second half
        for j in range(W):
            nc.gpsimd.dma_start(hT_c[:, j, :], a2a_out2b[j])
        emit_outproj(3, hT_c, 0)

    nc.compile()
    return nc


def _prep_inputs(x, Wq, bq, Wk, bk, Wv, bv, Wo, bo):
    xT = np.ascontiguousarray(x.reshape(TOK, D).T)
    in_maps = []
    for c in range(W):
        sl = slice(128 * c, 128 * (c + 1))
        bqkv = np.stack([bq[sl] / 8.0, bk[sl], bv[sl]], axis=1).astype(np.float32)
        in_maps.append({
            "xT": xT,
            "wq": np.ascontiguousarray(Wq[:, sl]) / 8.0,
            "wk": np.ascontiguousarray(Wk[:, sl]),
            "wv": np.ascontiguousarray(Wv[:, sl]),
            "wo": Wo,
            "bqkv": np.ascontiguousarray(bqkv),
        })
    return in_maps


def run(x, Wq, bq, Wk, bk, Wv, bv, Wo, bo, **run_kwargs):
    if "nc" not in _CACHE:
        _CACHE["nc"] = build_bass()
    nc = _CACHE["nc"]
    in_maps = _prep_inputs(x, Wq, bq, Wk, bk, Wv, bv, Wo, bo)
    res = run_bass_kernel_spmd(nc, in_maps, list(range(W)), **run_kwargs)
    out = np.empty((TOK, D), np.float32)
    for c in range(W):
        r = res.results[c]["out"]
        out[256 * c:256 * (c + 1)] = r[0:256]
        out[2048 + 128 * c:2048 + 128 * (c + 1)] = r[256:384]
        out[3072 + 128 * c:3072 + 128 * (c + 1)] = r[384:512]
    out = out.reshape(B, N, D) + bo.astype(np.float32)
    return out.astype(np.float32), res


def kernel(x, Wq, bq, Wk, bk, Wv, bv, Wo, bo):
    x, Wq, bq, Wk, bk, Wv, bv, Wo, bo = (
        np.asarray(a, dtype=np.float32)
        for a in (x, Wq, bq, Wk, bk, Wv, bv, Wo, bo)
    )
    out, _ = run(x, Wq, bq, Wk, bk, Wv, bv, Wo, bo)
    return out


# revision 20
# speedup vs baseline: 1.0096x; 1.0096x over previous
"""Multi-head attention forward on 8 Trainium2 NeuronCores (Bass/Tile).

Problem: B=2, N=2048, D=1024, H=16 heads of dh=64, fp32.

Sharding: tensor-parallel over heads — core c owns heads {2c, 2c+1} and both
batches for projections + attention; an on-device AllToAll then re-shards by
token so each core computes the output projection (full Wo) for its 512-token
slice with no reduction.

Layouts: all activations travel as [feature, token] ("transposed"), so every
matmul contraction lands on the partition axis:
  qT/kT/vT [128, 4096]  (rows 0-63 head A dims, 64-127 head B dims)
  scoresT[m, n] = kT.T @ qT   (softmax axis m = partitions)
  exp via ScalarE (no max subtraction: scores ~ N(0,1), exp is safe in fp32)
  attn@v: lhsT = v_aug [m, 65] (v transposed back per 128-chunk via PE
  transpose, with a ones column appended) so PSUM row 64 accumulates the
  softmax denominators for free.
  normalization: reciprocal of denom row, broadcast across partitions with a
  one-hot selector matmul, applied on VectorE.

Matmuls run in float32r (TF32-like, ~1.5e-4 rel err, full PE rate at free
dim >= 256). fp32 inputs are DMA'd directly into float32r tiles (legal when
the DRAM tensor is declared float32r).
"""
from contextlib import ExitStack

import numpy as np

import concourse.bass as bass
import concourse.tile as tile
from concourse import bacc, mybir
from concourse.bass_utils import run_bass_kernel_spmd
from concourse.masks import make_identity

F32 = mybir.dt.float32
F32R = mybir.dt.float32r

B, N, D, H, DH = 2, 2048, 1024, 16, 64
W = 8                    # cores
TOK = B * N              # 4096 flattened tokens
TPC = TOK // W           # 512 tokens per core after re-shard
HPC = H // W             # 2 heads per core

_CACHE = {}


def build_bass():
    nc = bacc.Bacc("TRN2", target_bir_lowering=False)

    xT_d = nc.declare_dram_parameter("xT", [D, TOK], F32R, isOutput=False)
    wq_d = nc.declare_dram_parameter("wq", [D, 128], F32R, isOutput=False)
    wk_d = nc.declare_dram_parameter("wk", [D, 128], F32R, isOutput=False)
    wv_d = nc.declare_dram_parameter("wv", [D, 128], F32R, isOutput=False)
    wo_d = nc.declare_dram_parameter("wo", [D, D], F32R, isOutput=False)
    bqkv_d = nc.declare_dram_parameter("bqkv", [128, 3], F32, isOutput=False)
    out_d = nc.declare_dram_parameter("out", [TPC, D], F32, isOutput=True)

    a2a_in1 = nc.dram_tensor("a2a_in1", [W, 128, 256], F32R)
    a2a_out1 = nc.dram_tensor("a2a_out1", [W, 128, 256], F32R)
    a2a_in2a = nc.dram_tensor("a2a_in2a", [W, 128, 128], F32R)
    a2a_out2a = nc.dram_tensor("a2a_out2a", [W, 128, 128], F32R)
    a2a_in2b = nc.dram_tensor("a2a_in2b", [W, 128, 128], F32R)
    a2a_out2b = nc.dram_tensor("a2a_out2b", [W, 128, 128], F32R)

    KC = D // 128        # contraction chunks for projections
    TC = TOK // 512      # 512-token chunks (8)
    MCB = N // 128       # m-chunks per batch (16)

    with tile.TileContext(nc) as tc, ExitStack() as ctx:
        sb1 = ctx.enter_context(tc.tile_pool(name="sb1", bufs=1))
        sbe = ctx.enter_context(tc.tile_pool(name="sbe", bufs=2))
        stage1 = ExitStack()
        sbw = stage1.enter_context(tc.tile_pool(name="sbw", bufs=1))
        sbx = stage1.enter_context(tc.tile_pool(name="sbx", bufs=2))
        ps_pj = stage1.enter_context(tc.tile_pool(name="ps_pj", bufs=2, space="PSUM"))

        # ---------- constants ----------
        ident_f = sb1.tile([128, 128], F32, tag="ident_f")
        make_identity(nc, ident_f[:])
        ident = sb1.tile([128, 128], F32R, tag="ident")
        nc.vector.tensor_copy(ident[:], ident_f[:])

        ones_f = sb1.tile([128, 1], F32, tag="ones_f")
        nc.vector.memset(ones_f[:], 1.0)
        ones_r = sb1.tile([128, 1], F32R, tag="ones_r")
        nc.vector.tensor_copy(ones_r[:], ones_f[:])

        sel = sb1.tile([128, 128], F32, tag="sel")
        nc.vector.memset(sel[:], 0.0)
        nc.vector.memset(sel[32:33, 0:64], 1.0)
        nc.vector.memset(sel[96:97, 64:128], 1.0)

        bias = sb1.tile([128, 3], F32, tag="bias")
        nc.sync.dma_start(bias[:], bqkv_d[:])

        zeros_f = sb1.tile([128, 512], F32, tag="zeros_f")
        nc.vector.memset(zeros_f[:], 0.0)
        zeros_r = sb1.tile([128, 512], F32R, tag="zeros_r")
        nc.vector.tensor_copy(zeros_r[:], zeros_f[:])

        # ---------- weights ----------
        wq = sbw.tile([128, KC, 128], F32R, tag="wq")
        wk = sbw.tile([128, KC, 128], F32R, tag="wk")
        wv = sbw.tile([128, KC, 128], F32R, tag="wv")
        for k in range(KC):
            nc.sync.dma_start(wq[:, k, :], wq_d[bass.ts(k, 128), :])

        # ---------- stage 1: projections (qT, kT resident; v -> v_aug) ----------
        qT = sb1.tile([128, TOK], F32R, tag="qT")
        # per-head kT, zero-padded to K=128 so score matmuls use the full
        # PE array (half-array shapes leave the clock gate cold)
        kT0p = sb1.tile([128, TOK], F32R, tag="kT0p")
        kT1p = sb1.tile([128, TOK], F32R, tag="kT1p")
        v_aug = sb1.tile([128, 2 * MCB, 130], F32R, tag="v_aug")

        for tp2 in range(TC // 2):
            ta, tb = 2 * tp2, 2 * tp2 + 1
            xta = sbx.tile([128, KC, 512], F32R, tag="xta")
            xtb = sbx.tile([128, KC, 512], F32R, tag="xtb")
            for k in range(KC):
                nc.sync.dma_start(xta[:, k, :], xT_d[bass.ts(k, 128), bass.ts(ta, 512)])
            for k in range(KC):
                nc.sync.dma_start(xtb[:, k, :], xT_d[bass.ts(k, 128), bass.ts(tb, 512)])
            if tp2 == 0:
                for k in range(KC):
                    nc.sync.dma_start(wk[:, k, :], wk_d[bass.ts(k, 128), :])
                    nc.sync.dma_start(wv[:, k, :], wv_d[bass.ts(k, 128), :])

            tsla, tslb = bass.ts(ta, 512), bass.ts(tb, 512)
            pja = ps_pj.tile([128, 512], F32, tag="pj0")
            pjb = ps_pj.tile([128, 512], F32, tag="pj1")
            for k in range(KC):
                nc.tensor.matmul(pja[:], wq[:, k, :], xta[:, k, :],
                                 start=(k == 0), stop=(k == KC - 1))
                nc.tensor.matmul(pjb[:], wq[:, k, :], xtb[:, k, :],
                                 start=(k == 0), stop=(k == KC - 1))
            nc.vector.tensor_scalar_add(qT[:, tsla], pja[:], bias[:, 0:1])
            nc.vector.tensor_scalar_add(qT[:, tslb], pjb[:], bias[:, 0:1])

            pja = ps_pj.tile([128, 512], F32, tag="pj0")
            pjb = ps_pj.tile([128, 512], F32, tag="pj1")
            for k in range(KC):
                nc.tensor.matmul(pja[:], wk[:, k, :], xta[:, k, :],
                                 start=(k == 0), stop=(k == KC - 1))
                nc.tensor.matmul(pjb[:], wk[:, k, :], xtb[:, k, :],
                                 start=(k == 0), stop=(k == KC - 1))
            for tsl, pj in ((tsla, pja), (tslb, pjb)):
                nc.vector.tensor_scalar_add(kT0p[0:64, tsl], pj[0:64, :], bias[0:64, 1:2])
                nc.vector.tensor_scalar_add(kT1p[64:128, tsl], pj[64:128, :], bias[64:128, 1:2])
                nc.vector.tensor_copy(kT0p[64:128, tsl], zeros_r[64:128, :])
                nc.vector.tensor_copy(kT1p[0:64, tsl], zeros_r[0:64, :])

            pja = ps_pj.tile([128, 512], F32, tag="pj0")
            pjb = ps_pj.tile([128, 512], F32, tag="pj1")
            for k in range(KC):
                nc.tensor.matmul(pja[:], wv[:, k, :], xta[:, k, :],
                                 start=(k == 0), stop=(k == KC - 1))
                nc.tensor.matmul(pjb[:], wv[:, k, :], xtb[:, k, :],
                                 start=(k == 0), stop=(k == KC - 1))
            vts = []
            for t, pj in ((ta, pja), (tb, pjb)):
                vt = sbx.tile([128, 512], F32R, tag=f"vt{t % 2}")
                nc.vector.tensor_scalar_add(vt[:], pj[:], bias[:, 2:3])
                vts.append((t, vt))
            # transpose v into v_aug rows (4 m-chunks per 512-token group)
            for t, vt in vts:
                for i in range(4):
                    gm = 4 * t + i
                    tp = ps_pj.tile([128, 128], F32R, tag="tp")
                    nc.tensor.transpose(tp[:], vt[:, bass.ts(i, 128)], ident[:])
                    nc.vector.tensor_copy(v_aug[:, gm, 0:64], tp[:, 0:64])
                    nc.vector.tensor_copy(v_aug[:, gm, 65:129], tp[:, 64:128])
                    nc.vector.tensor_copy(v_aug[:, gm, 64:65], ones_r[:])
                    nc.vector.tensor_copy(v_aug[:, gm, 129:130], ones_r[:])

        stage1.close()
        sb3 = ctx.enter_context(tc.tile_pool(name="sb3", bufs=1))
        wo = sb3.tile([128, KC, D], F32R, tag="wo")
        for k in range(KC):
            nc.sync.dma_start(wo[:, k, :], wo_d[bass.ts(k, 128), :])
        # ---------- stage 2: attention ----------
        stage2 = ExitStack()
        ps_sc = stage2.enter_context(tc.tile_pool(name="ps_sc", bufs=1, space="PSUM"))
        ps_ha = stage2.enter_context(tc.tile_pool(name="ps_ha", bufs=1, space="PSUM"))
        heads = sb1.tile([128, TOK], F32R, tag="heads")
        rcp = sb1.tile([128, 1024], F32, tag="rcp")
        nc.vector.memset(rcp[:], 0.0)

        def emit_a2a(a_in, a_out, col0, width):
            for j in range(W):
                nc.sync.dma_start(a_in[j], heads[:, bass.ds(col0 + width * j, width)])
            nc.gpsimd.collective_compute(
                "AllToAll",
                mybir.AluOpType.bypass,
                ins=[a_in[:]],
                outs=[a_out[:]],
                replica_groups=[list(range(W))],
            )

        def emit_normalize(pend, bc_pool=None, bc_tag="sc0"):
            # selector matmul broadcasts 1/denom across partitions, then
            # VectorE applies it; emitted one window late so the PE-queue
            # stall on the reciprocal DMA round-trip hides inside the next
            # window's matmul stream.
            hs0, hs1, ptok0, pb, pnh = pend
            pool = bc_pool if bc_pool is not None else ps_sc
            for q4 in range(2):
                psl = bass.ts(q4, 512)
                bc = pool.tile([128, 512], F32, tag=bc_tag)
                nc.tensor.matmul(bc[:], sel[:], rcp[:, psl], start=True, stop=True)
                bc_s = sbe.tile([128, 512], F32, tag="bc_s", bufs=1)
                nc.vector.reciprocal_approx_fast(bc_s[:], bc[:])
                hsl = bass.ds(ptok0 + 512 * q4, 512)
                nc.vector.tensor_mul(heads[0:64, hsl], hs0[0:64, psl], bc_s[0:64, :])
                nc.vector.tensor_mul(heads[64:128, hsl], hs1[64:128, psl], bc_s[64:128, :])
            if (pb, pnh) == (0, 1):
                emit_a2a(a2a_in1, a2a_out1, 0, 256)       # batch-0 heads
            elif (pb, pnh) == (1, 0):
                emit_a2a(a2a_in2a, a2a_out2a, 2048, 128)  # batch-1 first half

        pending = None
        for b in range(B):
            for nh in range(2):                   # 1024-token n-window
                tok0 = 2048 * b + 1024 * nh
                ha0 = ps_ha.tile([65, 1024], F32, tag="ha0")
                ha1 = ps_ha.tile([65, 1024], F32, tag="ha1")
                # software pipeline: attn@v for m-chunk mc-1 runs alongside
                # scores/exp for mc, so PE never waits on a fresh exp.
                prev = None
                for mc in range(MCB):
                    gm = MCB * b + mc
                    msl = bass.ts(gm, 128)
                    sc0 = ps_sc.tile([128, 1024], F32, tag="sc0")
                    sc1 = ps_sc.tile([128, 1024], F32, tag="sc1")
                    for q4 in range(2):
                        nsl = bass.ds(tok0 + 512 * q4, 512)
                        psl = bass.ts(q4, 512)
                        nc.tensor.matmul(sc0[:, psl], kT0p[:, msl], qT[:, nsl],
                                         start=True, stop=True)
                        nc.tensor.matmul(sc1[:, psl], kT1p[:, msl], qT[:, nsl],
                                         start=True, stop=True)
                    if prev is not None:
                        pe0, pe1, pgm = prev
                        for q4 in range(2):
                            psl = bass.ts(q4, 512)
                            nc.tensor.matmul(ha0[:, psl], v_aug[:, pgm, 0:65],
                                             pe0[:, psl], start=(pgm % MCB == 0), stop=False)
                            nc.tensor.matmul(ha1[:, psl], v_aug[:, pgm, 65:130],
                                             pe1[:, psl], start=(pgm % MCB == 0), stop=False)
                    e0 = sbe.tile([128, 1024], F32R, tag="e0")
                    e1 = sbe.tile([128, 1024], F32R, tag="e1")
                    nc.scalar.activation(e0[:], sc0[:], mybir.ActivationFunctionType.Exp)
                    nc.scalar.activation(e1[:], sc1[:], mybir.ActivationFunctionType.Exp)
                    prev = (e0, e1, gm)
                    if mc == 3 and pending is not None:
                        emit_normalize(pending)
                        pending = None
                # epilogue: last m-chunk's attn@v
                pe0, pe1, pgm = prev
                for q4 in range(2):
                    psl = bass.ts(q4, 512)
                    nc.tensor.matmul(ha0[:, psl], v_aug[:, pgm, 0:65], pe0[:, psl],
                                     start=False, stop=True)
                    nc.tensor.matmul(ha1[:, psl], v_aug[:, pgm, 65:130], pe1[:, psl],
                                     start=False, stop=True)

                # free the ha PSUM banks quickly: copy to SBUF, then compute
                # reciprocals off the PE queue (VectorE + DMA reshape).
                hs0 = sbe.tile([65, 1024], F32, tag="hs0", bufs=1)
                hs1 = sbe.tile([128, 1024], F32, tag="hs1", bufs=1)
                nc.vector.tensor_copy(hs0[:], ha0[:])
                nc.vector.tensor_copy(hs1[64:128, :], ha1[0:64, :])
                nc.vector.tensor_copy(rcp[32:33, :], hs0[64:65, :])
                nc.vector.tensor_copy(rcp[96:97, :], ha1[64:65, :])
                pending = (hs0, hs1, tok0, b, nh)

        stage2.close()
        # ---------- stage 3: output projection on re-sharded tokens ----------
        # hT is split into one tile per collective so the out-proj matmuls for
        # already-received tokens don't falsely wait on later collectives.
        ps_op = ctx.enter_context(tc.tile_pool(name="ps_op", bufs=2, space="PSUM"))
        hT_a = sb3.tile([128, W, 256], F32R, tag="hT_a")
        hT_b = sb3.tile([128, W, 128], F32R, tag="hT_b")
        hT_c = sb3.tile([128, W, 128], F32R, tag="hT_c")

        def emit_outproj(tq, src_t, col0):
            for dc in range(2):
                op = ps_op.tile([128, 512], F32, tag="op")
                for k in range(KC):
                    nc.tensor.matmul(op[:], src_t[:, k, bass.ds(col0, 128)],
                                     wo[:, k, bass.ts(dc, 512)],
                                     start=(k == 0), stop=(k == KC - 1))
                ot = sb3.tile([128, 512], F32, tag="ot", bufs=2)
                # ScalarE copy: VectorE's queue is busy with the normalize
                # chain at the tail and would head-of-line block these
                nc.scalar.activation(ot[:], op[:], mybir.ActivationFunctionType.Copy)
                nc.sync.dma_start(out_d[bass.ts(tq, 128), bass.ts(dc, 512)], ot[:])

        # tokens already received (batch 0 + batch-1 first half) project while
        # the final window's normalize chain and last collective run
        for j in range(W):
            nc.sync.dma_start(hT_a[:, j, :], a2a_out1[j])
        emit_outproj(0, hT_a, 0)
        emit_outproj(1, hT_a, 128)
        for j in range(W):
            nc.gpsimd.dma_start(hT_b[:, j, :], a2a_out2a[j])
        emit_outproj(2, hT_b, 0)

        emit_normalize(pending, bc_pool=ps_op, bc_tag="op")
        pending = None
        emit_a2a(a2a_in2b, a2a_out2b, 3072, 128)          # batch-1 second half
        for j in range(W):
            nc.gpsimd.dma_start(hT_c[:, j, :], a2a_out2b[j])
        emit_outproj(3, hT_c, 0)

    nc.compile()
    return nc


def _prep_inputs(x, Wq, bq, Wk, bk, Wv, bv, Wo, bo):
    xT = np.ascontiguousarray(x.reshape(TOK, D).T)
    in_maps = []
    for c in range(W):
        sl = slice(128 * c, 128 * (c + 1))
        bqkv = np.stack([bq[sl] / 8.0, bk[sl], bv[sl]], axis=1).astype(np.float32)
        in_maps.append({
            "xT": xT,
            "wq": np.ascontiguousarray(Wq[:, sl]) / 8.0,
            "wk": np.ascontiguousarray(Wk[:, sl]),
            "wv": np.ascontiguousarray(Wv[:, sl]),
            "wo": Wo,
            "bqkv": np.ascontiguousarray(bqkv),
        })
    return in_maps


def run(x, Wq, bq, Wk, bk, Wv, bv, Wo, bo, **run_kwargs):
    if "nc" not in _CACHE:
        _CACHE["nc"] = build_bass()
    nc = _CACHE["nc"]
    in_maps = _prep_inputs(x, Wq, bq, Wk, bk, Wv, bv, Wo, bo)
    res = run_bass_kernel_spmd(nc, in_maps, list(range(W)), **run_kwargs)
    out = np.empty((TOK, D), np.float32)
    for c in range(W):
        r = res.results[c]["out"]
        out[256 * c:256 * (c + 1)] = r[0:256]
        out[2048 + 128 * c:2048 + 128 * (c + 1)] = r[256:384]
        out[3072 + 128 * c:3072 + 128 * (c + 1)] = r[384:512]
    out = out.reshape(B, N, D) + bo.astype(np.float32)
    return out.astype(np.float32), res


def kernel(x, Wq, bq, Wk, bk, Wv, bv, Wo, bo):
    x, Wq, bq, Wk, bk, Wv, bv, Wo, bo = (
        np.asarray(a, dtype=np.float32)
        for a in (x, Wq, bq, Wk, bk, Wv, bv, Wo, bo)
    )
    out, _ = run(x, Wq, bq, Wk, bk, Wv, bv, Wo, bo)
    return out


# revision 21
# speedup vs baseline: 1.0354x; 1.0255x over previous
"""Multi-head attention forward on 8 Trainium2 NeuronCores (Bass/Tile).

Problem: B=2, N=2048, D=1024, H=16 heads of dh=64, fp32.

Sharding: tensor-parallel over heads — core c owns heads {2c, 2c+1} and both
batches for projections + attention; an on-device AllToAll then re-shards by
token so each core computes the output projection (full Wo) for its 512-token
slice with no reduction.

Layouts: all activations travel as [feature, token] ("transposed"), so every
matmul contraction lands on the partition axis:
  qT/kT/vT [128, 4096]  (rows 0-63 head A dims, 64-127 head B dims)
  scoresT[m, n] = kT.T @ qT   (softmax axis m = partitions)
  exp via ScalarE (no max subtraction: scores ~ N(0,1), exp is safe in fp32)
  attn@v: lhsT = v_aug [m, 65] (v transposed back per 128-chunk via PE
  transpose, with a ones column appended) so PSUM row 64 accumulates the
  softmax denominators for free.
  normalization: reciprocal of denom row, broadcast across partitions with a
  one-hot selector matmul, applied on VectorE.

Matmuls run in float32r (TF32-like, ~1.5e-4 rel err, full PE rate at free
dim >= 256). fp32 inputs are DMA'd directly into float32r tiles (legal when
the DRAM tensor is declared float32r).
"""
from contextlib import ExitStack

import numpy as np

import concourse.bass as bass
import concourse.tile as tile
from concourse import bacc, mybir
from concourse.bass_utils import run_bass_kernel_spmd
from concourse.masks import make_identity

F32 = mybir.dt.float32
F32R = mybir.dt.float32r

B, N, D, H, DH = 2, 2048, 1024, 16, 64
W = 8                    # cores
TOK = B * N              # 4096 flattened tokens
TPC = TOK // W           # 512 tokens per core after re-shard
HPC = H // W             # 2 heads per core

_CACHE = {}


def build_bass():
    nc = bacc.Bacc("TRN2", target_bir_lowering=False)

    xT_d = nc.declare_dram_parameter("xT", [D, TOK], F32R, isOutput=False)
    wq_d = nc.declare_dram_parameter("wq", [D, 128], F32R, isOutput=False)
    wk_d = nc.declare_dram_parameter("wk", [D, 128], F32R, isOutput=False)
    wv_d = nc.declare_dram_parameter("wv", [D, 128], F32R, isOutput=False)
    wo_d = nc.declare_dram_parameter("wo", [D, D], F32R, isOutput=False)
    bqkv_d = nc.declare_dram_parameter("bqkv", [128, 3], F32, isOutput=False)
    out_d = nc.declare_dram_parameter("out", [TPC, D], F32, isOutput=True)

    a2a_in1 = nc.dram_tensor("a2a_in1", [W, 128, 256], F32R)
    a2a_out1 = nc.dram_tensor("a2a_out1", [W, 128, 256], F32R)
    a2a_in2a = nc.dram_tensor("a2a_in2a", [W, 128, 128], F32R)
    a2a_out2a = nc.dram_tensor("a2a_out2a", [W, 128, 128], F32R)
    a2a_in2b = nc.dram_tensor("a2a_in2b", [W, 128, 128], F32R)
    a2a_out2b = nc.dram_tensor("a2a_out2b", [W, 128, 128], F32R)

    KC = D // 128        # contraction chunks for projections
    TC = TOK // 512      # 512-token chunks (8)
    MCB = N // 128       # m-chunks per batch (16)

    with tile.TileContext(nc) as tc, ExitStack() as ctx:
        sb1 = ctx.enter_context(tc.tile_pool(name="sb1", bufs=1))
        sbe = ctx.enter_context(tc.tile_pool(name="sbe", bufs=2))
        stage1 = ExitStack()
        sbw = stage1.enter_context(tc.tile_pool(name="sbw", bufs=1))
        sbx = stage1.enter_context(tc.tile_pool(name="sbx", bufs=2))
        ps_pj = stage1.enter_context(tc.tile_pool(name="ps_pj", bufs=2, space="PSUM"))

        # ---------- constants ----------
        ident_f = sb1.tile([128, 128], F32, tag="ident_f")
        make_identity(nc, ident_f[:])
        ident = sb1.tile([128, 128], F32R, tag="ident")
        nc.vector.tensor_copy(ident[:], ident_f[:])

        ones_f = sb1.tile([128, 1], F32, tag="ones_f")
        nc.vector.memset(ones_f[:], 1.0)
        ones_r = sb1.tile([128, 1], F32R, tag="ones_r")
        nc.vector.tensor_copy(ones_r[:], ones_f[:])

        sel = sb1.tile([128, 128], F32, tag="sel")
        nc.vector.memset(sel[:], 0.0)
        nc.vector.memset(sel[32:33, 0:64], 1.0)
        nc.vector.memset(sel[96:97, 64:128], 1.0)

        bias = sb1.tile([128, 3], F32, tag="bias")
        nc.sync.dma_start(bias[:], bqkv_d[:])

        zeros_f = sb1.tile([128, 512], F32, tag="zeros_f")
        nc.vector.memset(zeros_f[:], 0.0)
        zeros_r = sb1.tile([128, 512], F32R, tag="zeros_r")
        nc.vector.tensor_copy(zeros_r[:], zeros_f[:])

        # ---------- weights ----------
        wq = sbw.tile([128, KC, 128], F32R, tag="wq")
        wk = sbw.tile([128, KC, 128], F32R, tag="wk")
        wv = sbw.tile([128, KC, 128], F32R, tag="wv")
        for k in range(KC):
            nc.sync.dma_start(wq[:, k, :], wq_d[bass.ts(k, 128), :])

        # ---------- stage 1: projections (qT, kT resident; v -> v_aug) ----------
        qT = sb1.tile([128, TOK], F32R, tag="qT")
        # per-head kT, zero-padded to K=128 so score matmuls use the full
        # PE array (half-array shapes leave the clock gate cold)
        kT0p = sb1.tile([128, TOK], F32R, tag="kT0p")
        kT1p = sb1.tile([128, TOK], F32R, tag="kT1p")
        v_aug = sb1.tile([128, 2 * MCB, 130], F32R, tag="v_aug")

        for tp2 in range(TC // 2):
            ta, tb = 2 * tp2, 2 * tp2 + 1
            xta = sbx.tile([128, KC, 512], F32R, tag="xta")
            xtb = sbx.tile([128, KC, 512], F32R, tag="xtb")
            for k in range(KC):
                nc.sync.dma_start(xta[:, k, :], xT_d[bass.ts(k, 128), bass.ts(ta, 512)])
            for k in range(KC):
                nc.sync.dma_start(xtb[:, k, :], xT_d[bass.ts(k, 128), bass.ts(tb, 512)])
            if tp2 == 0:
                for k in range(KC):
                    nc.sync.dma_start(wk[:, k, :], wk_d[bass.ts(k, 128), :])
                    nc.sync.dma_start(wv[:, k, :], wv_d[bass.ts(k, 128), :])

            tsla, tslb = bass.ts(ta, 512), bass.ts(tb, 512)
            pja = ps_pj.tile([128, 512], F32, tag="pj0")
            pjb = ps_pj.tile([128, 512], F32, tag="pj1")
            for k in range(KC):
                nc.tensor.matmul(pja[:], wq[:, k, :], xta[:, k, :],
                                 start=(k == 0), stop=(k == KC - 1))
                nc.tensor.matmul(pjb[:], wq[:, k, :], xtb[:, k, :],
                                 start=(k == 0), stop=(k == KC - 1))
            nc.vector.tensor_scalar_add(qT[:, tsla], pja[:], bias[:, 0:1])
            nc.vector.tensor_scalar_add(qT[:, tslb], pjb[:], bias[:, 0:1])

            pja = ps_pj.tile([128, 512], F32, tag="pj0")
            pjb = ps_pj.tile([128, 512], F32, tag="pj1")
            for k in range(KC):
                nc.tensor.matmul(pja[:], wk[:, k, :], xta[:, k, :],
                                 start=(k == 0), stop=(k == KC - 1))
                nc.tensor.matmul(pjb[:], wk[:, k, :], xtb[:, k, :],
                                 start=(k == 0), stop=(k == KC - 1))
            for tsl, pj in ((tsla, pja), (tslb, pjb)):
                nc.vector.tensor_scalar_add(kT0p[0:64, tsl], pj[0:64, :], bias[0:64, 1:2])
                nc.vector.tensor_scalar_add(kT1p[64:128, tsl], pj[64:128, :], bias[64:128, 1:2])
                nc.vector.tensor_copy(kT0p[64:128, tsl], zeros_r[64:128, :])
                nc.vector.tensor_copy(kT1p[0:64, tsl], zeros_r[0:64, :])

            pja = ps_pj.tile([128, 512], F32, tag="pj0")
            pjb = ps_pj.tile([128, 512], F32, tag="pj1")
            for k in range(KC):
                nc.tensor.matmul(pja[:], wv[:, k, :], xta[:, k, :],
                                 start=(k == 0), stop=(k == KC - 1))
                nc.tensor.matmul(pjb[:], wv[:, k, :], xtb[:, k, :],
                                 start=(k == 0), stop=(k == KC - 1))
            vts = []
            for t, pj in ((ta, pja), (tb, pjb)):
                vt = sbx.tile([128, 512], F32R, tag=f"vt{t % 2}")
                nc.vector.tensor_scalar_add(vt[:], pj[:], bias[:, 2:3])
                vts.append((t, vt))
            # transpose v into v_aug rows (4 m-chunks per 512-token group)
            for t, vt in vts:
                for i in range(4):
                    gm = 4 * t + i
                    tp = ps_pj.tile([128, 128], F32R, tag="tp")
                    nc.tensor.transpose(tp[:], vt[:, bass.ts(i, 128)], ident[:])
                    nc.vector.tensor_copy(v_aug[:, gm, 0:64], tp[:, 0:64])
                    nc.vector.tensor_copy(v_aug[:, gm, 65:129], tp[:, 64:128])
                    nc.vector.tensor_copy(v_aug[:, gm, 64:65], ones_r[:])
                    nc.vector.tensor_copy(v_aug[:, gm, 129:130], ones_r[:])

        stage1.close()
        sb3 = ctx.enter_context(tc.tile_pool(name="sb3", bufs=1))
        wo = sb3.tile([128, KC, D], F32R, tag="wo")
        for k in range(KC):
            nc.sync.dma_start(wo[:, k, :], wo_d[bass.ts(k, 128), :])
        # ---------- stage 2: attention ----------
        stage2 = ExitStack()
        ps_sc = stage2.enter_context(tc.tile_pool(name="ps_sc", bufs=1, space="PSUM"))
        ps_ha = stage2.enter_context(tc.tile_pool(name="ps_ha", bufs=1, space="PSUM"))
        heads = sb1.tile([128, TOK], F32R, tag="heads")
        rcp = sb1.tile([128, 1024], F32, tag="rcp")
        nc.vector.memset(rcp[:], 0.0)

        def emit_a2a(a_in, a_out, col0, width):
            for j in range(W):
                nc.sync.dma_start(a_in[j], heads[:, bass.ds(col0 + width * j, width)])
            nc.gpsimd.collective_compute(
                "AllToAll",
                mybir.AluOpType.bypass,
                ins=[a_in[:]],
                outs=[a_out[:]],
                replica_groups=[list(range(W))],
            )

        def emit_normalize_half(pend, q4, bc_pool=None, bc_tag="sc0"):
            # selector matmul broadcasts the denominator across partitions,
            # one approx-reciprocal turns it into 1/denom, VectorE applies it;
            # emitted one window late so it hides inside the next window's
            # matmul stream.
            hs0, hs1, ptok0, pb, pnh = pend
            pool = bc_pool if bc_pool is not None else ps_sc
            psl = bass.ts(q4, 512)
            bc = pool.tile([128, 512], F32, tag=bc_tag)
            nc.tensor.matmul(bc[:], sel[:], rcp[:, psl], start=True, stop=True)
            bc_s = sbe.tile([128, 512], F32, tag="bc_s", bufs=1)
            nc.vector.reciprocal_approx_fast(bc_s[:], bc[:])
            hsl = bass.ds(ptok0 + 512 * q4, 512)
            nc.vector.tensor_mul(heads[0:64, hsl], hs0[0:64, psl], bc_s[0:64, :])
            nc.vector.tensor_mul(heads[64:128, hsl], hs1[64:128, psl], bc_s[64:128, :])

        def emit_normalize(pend, bc_pool=None, bc_tag="sc0"):
            hs0, hs1, ptok0, pb, pnh = pend
            for q4 in range(2):
                emit_normalize_half(pend, q4, bc_pool, bc_tag)
            emit_ship(pend)

        def emit_ship(pend):
            hs0, hs1, ptok0, pb, pnh = pend
            if (pb, pnh) == (0, 1):
                emit_a2a(a2a_in1, a2a_out1, 0, 256)       # batch-0 heads
            elif (pb, pnh) == (1, 0):
                emit_a2a(a2a_in2a, a2a_out2a, 2048, 128)  # batch-1 first half

        pending = None
        for b in range(B):
            for nh in range(2):                   # 1024-token n-window
                tok0 = 2048 * b + 1024 * nh
                ha0 = ps_ha.tile([65, 1024], F32, tag="ha0")
                ha1 = ps_ha.tile([65, 1024], F32, tag="ha1")
                # software pipeline: attn@v for m-chunk mc-1 runs alongside
                # scores/exp for mc, so PE never waits on a fresh exp.
                prev = None
                for mc in range(MCB):
                    gm = MCB * b + mc
                    msl = bass.ts(gm, 128)
                    sc0 = ps_sc.tile([128, 1024], F32, tag="sc0")
                    sc1 = ps_sc.tile([128, 1024], F32, tag="sc1")
                    for q4 in range(2):
                        nsl = bass.ds(tok0 + 512 * q4, 512)
                        psl = bass.ts(q4, 512)
                        nc.tensor.matmul(sc0[:, psl], kT0p[:, msl], qT[:, nsl],
                                         start=True, stop=True)
                        nc.tensor.matmul(sc1[:, psl], kT1p[:, msl], qT[:, nsl],
                                         start=True, stop=True)
                    if prev is not None:
                        pe0, pe1, pgm = prev
                        for q4 in range(2):
                            psl = bass.ts(q4, 512)
                            nc.tensor.matmul(ha0[:, psl], v_aug[:, pgm, 0:65],
                                             pe0[:, psl], start=(pgm % MCB == 0), stop=False)
                            nc.tensor.matmul(ha1[:, psl], v_aug[:, pgm, 65:130],
                                             pe1[:, psl], start=(pgm % MCB == 0), stop=False)
                    e0 = sbe.tile([128, 1024], F32R, tag="e0")
                    e1 = sbe.tile([128, 1024], F32R, tag="e1")
                    nc.scalar.activation(e0[:], sc0[:], mybir.ActivationFunctionType.Exp)
                    nc.scalar.activation(e1[:], sc1[:], mybir.ActivationFunctionType.Exp)
                    prev = (e0, e1, gm)
                    if mc == 3 and pending is not None:
                        emit_normalize_half(pending, 0)
                    if mc == 6 and pending is not None:
                        emit_normalize_half(pending, 1)
                        emit_ship(pending)
                        pending = None
                # epilogue: last m-chunk's attn@v
                pe0, pe1, pgm = prev
                for q4 in range(2):
                    psl = bass.ts(q4, 512)
                    nc.tensor.matmul(ha0[:, psl], v_aug[:, pgm, 0:65], pe0[:, psl],
                                     start=False, stop=True)
                    nc.tensor.matmul(ha1[:, psl], v_aug[:, pgm, 65:130], pe1[:, psl],
                                     start=False, stop=True)

                # free the ha PSUM banks quickly: copy to SBUF, then compute
                # reciprocals off the PE queue (VectorE + DMA reshape).
                hs0 = sbe.tile([65, 1024], F32, tag="hs0", bufs=1)
                hs1 = sbe.tile([128, 1024], F32, tag="hs1", bufs=1)
                nc.vector.tensor_copy(hs0[:], ha0[:])
                nc.vector.tensor_copy(hs1[64:128, :], ha1[0:64, :])
                nc.vector.tensor_copy(rcp[32:33, :], hs0[64:65, :])
                nc.vector.tensor_copy(rcp[96:97, :], ha1[64:65, :])
                pending = (hs0, hs1, tok0, b, nh)

        stage2.close()
        # ---------- stage 3: output projection on re-sharded tokens ----------
        # hT is split into one tile per collective so the out-proj matmuls for
        # already-received tokens don't falsely wait on later collectives.
        ps_op = ctx.enter_context(tc.tile_pool(name="ps_op", bufs=2, space="PSUM"))
        hT_a = sb3.tile([128, W, 256], F32R, tag="hT_a")
        hT_b = sb3.tile([128, W, 128], F32R, tag="hT_b")
        hT_c = sb3.tile([128, W, 128], F32R, tag="hT_c")

        def emit_outproj(tq, src_t, col0):
            for dc in range(2):
                op = ps_op.tile([128, 512], F32, tag="op")
                for k in range(KC):
                    nc.tensor.matmul(op[:], src_t[:, k, bass.ds(col0, 128)],
                                     wo[:, k, bass.ts(dc, 512)],
                                     start=(k == 0), stop=(k == KC - 1))
                ot = sb3.tile([128, 512], F32, tag="ot", bufs=2)
                # ScalarE copy: VectorE's queue is busy with the normalize
                # chain at the tail and would head-of-line block these
                nc.scalar.activation(ot[:], op[:], mybir.ActivationFunctionType.Copy)
                nc.sync.dma_start(out_d[bass.ts(tq, 128), bass.ts(dc, 512)], ot[:])

        # tokens already received (batch 0 + batch-1 first half) project while
        # the final window's normalize chain and last collective run
        for j in range(W):
            nc.sync.dma_start(hT_a[:, j, :], a2a_out1[j])
        emit_outproj(0, hT_a, 0)
        emit_outproj(1, hT_a, 128)
        for j in range(W):
            eng = nc.gpsimd if j % 2 == 0 else nc.scalar
            eng.dma_start(hT_b[:, j, :], a2a_out2a[j])
        emit_outproj(2, hT_b, 0)

        emit_normalize(pending, bc_pool=ps_op, bc_tag="op")
        pending = None
        emit_a2a(a2a_in2b, a2a_out2b, 3072, 128)          # batch-1 second half
        for j in range(W):
            eng = nc.gpsimd if j % 2 == 0 else nc.scalar
            eng.dma_start(hT_c[:, j, :], a2a_out2b[j])
        emit_outproj(3, hT_c, 0)

    nc.compile()
    return nc


def _prep_inputs(x, Wq, bq, Wk, bk, Wv, bv, Wo, bo):
    xT = np.ascontiguousarray(x.reshape(TOK, D).T)
    in_maps = []
    for c in range(W):
        sl = slice(128 * c, 128 * (c + 1))
        bqkv = np.stack([bq[sl] / 8.0, bk[sl], bv[sl]], axis=1).astype(np.float32)
        in_maps.append({
            "xT": xT,
            "wq": np.ascontiguousarray(Wq[:, sl]) / 8.0,
            "wk": np.ascontiguousarray(Wk[:, sl]),
            "wv": np.ascontiguousarray(Wv[:, sl]),
            "wo": Wo,
            "bqkv": np.ascontiguousarray(bqkv),
        })
    return in_maps


def run(x, Wq, bq, Wk, bk, Wv, bv, Wo, bo, **run_kwargs):
    if "nc" not in _CACHE:
        _CACHE["nc"] = build_bass()
    nc = _CACHE["nc"]
    in_maps = _prep_inputs(x, Wq, bq, Wk, bk, Wv, bv, Wo, bo)
    res = run_bass_kernel_spmd(nc, in_maps, list(range(W)), **run_kwargs)
    out = np.empty((TOK, D), np.float32)
    for c in range(W):
        r = res.results[c]["out"]
        out[256 * c:256 * (c + 1)] = r[0:256]
        out[2048 + 128 * c:2048 + 128 * (c + 1)] = r[256:384]
        out[3072 + 128 * c:3072 + 128 * (c + 1)] = r[384:512]
    out = out.reshape(B, N, D) + bo.astype(np.float32)
    return out.astype(np.float32), res


def kernel(x, Wq, bq, Wk, bk, Wv, bv, Wo, bo):
    x, Wq, bq, Wk, bk, Wv, bv, Wo, bo = (
        np.asarray(a, dtype=np.float32)
        for a in (x, Wq, bq, Wk, bk, Wv, bv, Wo, bo)
    )
    out, _ = run(x, Wq, bq, Wk, bk, Wv, bv, Wo, bo)
    return out
